# revision 22
# baseline (speedup 1.0000x reference)
"""GQA attention (16 Q heads / 4 KV heads, RoPE, n=2048, d=64) on 8 trn2 cores.

Sharding: core c = (batch b=c//4, kv-group j=c%4). Each core owns 4 query
heads sharing one KV head, computes its partial output projection
(O_heads @ Wo_rows), and the host sums the 4 partials per batch.

v3 design (ACT-exp is the per-core engine floor at ~153us):
  - warmup matmuls at t=0 flip the HAM clock gate before real work arrives;
    a dummy exp preloads the ACT spline table.
  - ch-major x DMA with per-ch rope-table slices; per 512-position chunk:
    KV proj -> cast to bf16 -> RoPE in 2x-mode bf16 -> kt_dup rows 0:64 and
    64:128; Q proj similarly into qt[pair] (head-even rows 0:64, head-odd
    64:128). V^T transposed from rows 64:128 via a stacked identity.
  - attention per (pair, 512-query chunk, key block): 2 row-tiled CONCURRENT
    S matmuls (K=64 each, full PE array), one N=1024 exp on ACT, 2 AV
    matmuls (65-col V with ones column -> denominators).
  - normalize off the critical path: 1 DVE copy evacuates PSUM (psO bufs=1),
    plain reciprocal (custom-DVE fast variant breaks without BIR lowering),
    gpsimd partition_broadcast, 2 muls.
  - chunk order (0,0),(0,1),(1,0),(1,1),(0,2),(1,2),(0,3),(1,3); out-proj
    units interleave only into chunks whose normalize-deps are >=1 chunk
    old (the tile scheduler models reciprocal as fast and otherwise hoists
    dependent LDWEIGHTS into the PE stream where they stall it).
"""

import os
import sys
import functools

import numpy as np

sys.path.insert(0, "/opt/trn_rl_repo")

import concourse.bass as bass  # noqa: E402
import concourse.bacc as bacc  # noqa: E402
import concourse.tile as tile  # noqa: E402
import concourse.mybir as mybir  # noqa: E402

F32 = mybir.dt.float32
BF16 = mybir.dt.bfloat16
EXP = mybir.ActivationFunctionType.Exp

B, N, DIM = 2, 2048, 1024
HEADS, KVH, D = 16, 4, 64
HPC = HEADS // KVH          # q heads per core = 4 (2 pairs)
SCALE = D ** -0.5           # 1/8
NKB = N // 128              # 16 key blocks
NDB = DIM // 128            # 8 contraction blocks for projections
NCH = 4                     # 512-position chunks

LAST_RESULTS = {}           # test.py introspection


def build_kernel(nc, tc, io):
    from contextlib import ExitStack

    xt, wq, wkv, wo = io["xt"], io["wq"], io["wkv"], io["wo"]
    cosq, sinq, out = io["cosq"], io["sinq"], io["out"]

    es = ExitStack()
    consts = es.enter_context(tc.tile_pool(name="consts", bufs=1))
    act = es.enter_context(tc.tile_pool(name="act", bufs=1))
    ropetmp = es.enter_context(tc.tile_pool(name="ropetmp", bufs=2))
    ppool = es.enter_context(tc.tile_pool(name="ppool", bufs=3))
    ostg = es.enter_context(tc.tile_pool(name="ostg", bufs=2))
    small = es.enter_context(tc.tile_pool(name="small", bufs=2))
    outstg = es.enter_context(tc.tile_pool(name="outstg", bufs=3))
    psS = es.enter_context(tc.tile_pool(name="psS", bufs=2, space="PSUM"))
    psO = es.enter_context(tc.tile_pool(name="psO", bufs=1, space="PSUM"))
    psT = es.enter_context(tc.tile_pool(name="psT", bufs=2, space="PSUM"))

    # --- constants / weights in SBUF ---
    wq_sb = consts.tile([128, 2, NDB, 128], BF16, tag="wq")       # 4KB/part
    wkv_sb = consts.tile([128, NDB, 128], BF16, tag="wkv")        # 2KB/part
    wo_sb = consts.tile([128, 2, DIM], BF16, tag="wo")            # 4KB/part
    cos_sb = consts.tile([128, N], BF16, tag="cos")               # 4KB/part
    sin_sb = consts.tile([128, N], BF16, tag="sin")               # 4KB/part
    scratch = consts.tile([128, 512], BF16, tag="scr")
    dummy = consts.tile([1, 8], F32, tag="dmy")

    # --- activations ---
    qt = [act.tile([128, N], BF16, tag=f"qt{p}", name=f"qt{p}") for p in (0, 1)]
    kt_dup = act.tile([128, N], BF16, tag="ktd")                  # K^T twice
    kvstage = act.tile([128, N], BF16, tag="kvst")                # K|V bf16
    vaug = act.tile([128, NKB, 128], BF16, tag="vaug")            # [keys, 65]
    ot = [act.tile([128, N], BF16, tag=f"ot{p}", name=f"ot{p}") for p in (0, 1)]

    # --- t=0: warm the PE clock gate + preload the exp table ---
    nc.vector.memset(scratch, 0.0)
    nc.scalar.activation(dummy, scratch[0:1, 0:8], EXP, bias=0.0, scale=1.0)
    for i in range(12):
        wps = psT.tile([128, 512], F32, tag="pt", name="wps")
        nc.tensor.matmul(wps, scratch[:, 0:128], scratch, start=True, stop=True)
    for ch in range(NCH):
        nc.vector.memset(vaug[:, 4 * ch:4 * ch + 4, 64:65], 1.0)

    # --- input DMAs (issue order matters: ch0 x first, then its tables) ---
    xt_sb = consts.tile([128, NCH, NDB, 512], BF16, tag="xt")     # 32KB/part

    def dma_xt(ch, nsplit):
        step = NDB // nsplit
        for s in range(nsplit):
            nc.sync.dma_start(
                xt_sb[:, ch, s * step:(s + 1) * step, :],
                xt[ch, s * step:(s + 1) * step].transpose([1, 0, 2]),
            )

    dma_xt(0, 4)                      # ch0 across 4 queues: lands first
    nc.sync.dma_start(wkv_sb, wkv.transpose([1, 0, 2]))
    dma_xt(1, 2)
    nc.sync.dma_start(cos_sb[:, 0:1024], cosq[:, 0:1024])
    nc.sync.dma_start(sin_sb[:, 0:1024], sinq[:, 0:1024])
    nc.sync.dma_start(wq_sb[:, 0], wq[0].transpose([1, 0, 2]))
    dma_xt(2, 2)
    nc.sync.dma_start(cos_sb[:, 1024:2048], cosq[:, 1024:2048])
    nc.sync.dma_start(sin_sb[:, 1024:2048], sinq[:, 1024:2048])
    nc.sync.dma_start(wq_sb[:, 1], wq[1].transpose([1, 0, 2]))
    dma_xt(3, 2)
    nc.sync.dma_start(wo_sb, wo.transpose([1, 0, 2]))

    def cc_cols(cc):
        return slice(cc * 512, (cc + 1) * 512)

    def proj_kv(ch):
        cols = cc_cols(ch)
        pkv = psT.tile([128, 512], F32, tag="pt", name="pkv")
        for kb in range(NDB):
            nc.tensor.matmul(
                pkv, wkv_sb[:, kb, :], xt_sb[:, ch, kb, :],
                start=(kb == 0), stop=(kb == NDB - 1),
            )
        # cast to bf16 (K rows 0:64 pre-rope staging, V rows 64:128)
        nc.vector.tensor_copy(kvstage[:, cols], pkv)
        t2 = ropetmp.tile([64, 512], BF16, tag="t2", name="t2k")
        nc.vector.tensor_mul(t2[0:32, :], kvstage[32:64, cols], sin_sb[32:64, cols])
        nc.vector.tensor_mul(t2[32:64, :], kvstage[0:32, cols], sin_sb[0:32, cols])
        t1 = ropetmp.tile([64, 512], BF16, tag="t1", name="t1k")
        nc.vector.tensor_mul(t1, kvstage[0:64, cols], cos_sb[0:64, cols])
        nc.vector.tensor_add(kt_dup[0:64, cols], t1, t2)
        nc.vector.tensor_add(kt_dup[64:128, cols], t1, t2)

    def vt_blocks(ch):
        # V_aug blocks via the DMA transpose XBAR (SBUF->SBUF, bf16): keeps
        # the PE, DVE and the shared psT slots out of the V transpose.
        for t in range(4 * ch, 4 * ch + 4):
            nc.sync.dma_start(
                vaug[:, t, 0:64], kvstage[64:128, t * 128:(t + 1) * 128],
                transpose=True,
            )

    def proj_q(pack, ch):
        cols = cc_cols(ch)
        pq = psT.tile([128, 512], F32, tag="pt", name="pq")
        for kb in range(NDB):
            nc.tensor.matmul(
                pq, wq_sb[:, pack, kb, :], xt_sb[:, ch, kb, :],
                start=(kb == 0), stop=(kb == NDB - 1),
            )
        qs = ropetmp.tile([128, 512], BF16, tag="qs", name="qs")
        nc.vector.tensor_copy(qs, pq)
        t2 = ropetmp.tile([128, 512], BF16, tag="t2q", name="t2q")
        for h in range(2):
            r = 64 * h
            nc.vector.tensor_mul(
                t2[r:r + 32, :], qs[r + 32:r + 64, :], sin_sb[r + 32:r + 64, cols]
            )
            nc.vector.tensor_mul(
                t2[r + 32:r + 64, :], qs[r:r + 32, :], sin_sb[r:r + 32, cols]
            )
        t1 = ropetmp.tile([128, 512], BF16, tag="t1q", name="t1q")
        nc.vector.tensor_mul(t1, qs, cos_sb[:, cols])
        nc.vector.tensor_add(qt[pack][:, cols], t1, t2)

    def attn_kbs(pair, cc, po, kbs):
        cols = cc_cols(cc)
        for kb in kbs:
            ps = psS.tile([128, 2, 512], F32, tag="s", name="ps")
            kcols = slice(kb * 128, (kb + 1) * 128)
            nc.tensor.matmul(
                ps[:, 0, :], kt_dup[0:64, kcols], qt[pair][0:64, cols],
                start=True, stop=True,
            )
            nc.tensor.matmul(
                ps[:, 1, :], kt_dup[64:128, kcols], qt[pair][64:128, cols],
                start=True, stop=True,
            )
            p = ppool.tile([128, 2, 512], BF16, tag="p", name="p")
            nc.scalar.activation(p, ps, EXP, bias=0.0, scale=SCALE)
            for h in range(2):
                nc.tensor.matmul(
                    po[0:65, h, :], vaug[:, kb, 0:65], p[:, h, :],
                    start=(kb == 0), stop=(kb == NKB - 1),
                    skip_group_check=True,
                )
        return p

    def finalize_evac(po):
        """Evacuate PSUM O fast so the psO slot frees for the next chunk."""
        os_t = ostg.tile([65, 2, 512], F32, tag="os", name="os_t")
        nc.vector.tensor_copy(os_t, po[0:65, :, :])
        return os_t

    def finalize_norm(pair, cc, os_t, half=None):
        """Normalize off the critical path.  1/Z via BITWISE_NOT seed + a
        Newton pass in STANDARD DVE ops (the fused custom op needs BIR
        lowering; nc.vector.reciprocal is ~6.4 cyc/elem but modeled at ~1 by
        the tile scheduler, which then mis-places dependent out-proj work).
        Seed err ~6% -> one Newton pass leaves <=0.4%, under bf16 noise.
        `half` (0/1) processes 256 of the 512 columns (tail-latency split)."""
        C0, C1 = -0.23549792, 2.0017324
        if half is None:
            qs_, w = slice(0, 512), 512
        else:
            qs_, w = slice(half * 256, (half + 1) * 256), 256
        cols = slice(cc * 512 + qs_.start, cc * 512 + qs_.stop)
        zr = small.tile([1, 2, 512], F32, tag="zr", name="zr", bufs=1)
        nc.vector.tensor_copy(zr[:, :, 0:w], os_t[64:65, :, qs_])
        nb = small.tile([1, 2, 512], mybir.dt.int32, tag="nb", name="nb", bufs=1)
        nc.vector.tensor_tensor(
            nb[:, :, 0:w], zr.bitcast(mybir.dt.int32)[:, :, 0:w],
            zr.bitcast(mybir.dt.int32)[:, :, 0:w], mybir.AluOpType.bitwise_not,
        )
        ya = small.tile([1, 2, 512], F32, tag="ya", name="ya", bufs=1)
        yb = small.tile([1, 2, 512], F32, tag="yb", name="yb", bufs=1)
        nc.vector.tensor_scalar_mul(ya[:, :, 0:w], nb.bitcast(F32)[:, :, 0:w], C0)
        nc.vector.tensor_mul(yb[:, :, 0:w], zr[:, :, 0:w], ya[:, :, 0:w])
        nc.vector.tensor_scalar(
            yb[:, :, 0:w], yb[:, :, 0:w], C1, -1.0,
            mybir.AluOpType.subtract, mybir.AluOpType.mult
        )                                                           # c1 - z*y0
        rec = small.tile([1, 2, 512], F32, tag="rec", name="rec")
        nc.vector.tensor_mul(rec[:, :, 0:w], ya[:, :, 0:w], yb[:, :, 0:w])
        bc = small.tile([64, 2, 512], F32, tag="bc", name="bc")
        nc.gpsimd.partition_broadcast(bc[:, :, 0:w], rec[:, :, 0:w])
        nc.vector.tensor_mul(
            ot[pair][0:64, cols], os_t[0:64, 0, qs_], bc[:, 0, 0:w]
        )
        nc.vector.tensor_mul(
            ot[pair][64:128, cols], os_t[0:64, 1, qs_], bc[:, 1, 0:w]
        )

    def outproj_unit(qb, nch):
        pt = psT.tile([128, 512], F32, tag="pt", name="pt")
        ocols = slice(nch * 512, (nch + 1) * 512)
        for pair in range(2):
            nc.tensor.matmul(
                pt, ot[pair][:, qb * 128:(qb + 1) * 128], wo_sb[:, pair, ocols],
                start=(pair == 0), stop=(pair == 1),
            )
        st = outstg.tile([128, 512], F32, tag="ost", name="st")
        nc.vector.tensor_copy(st, pt)
        nc.sync.dma_start(out[qb * 128:(qb + 1) * 128, ocols], st)

    # --- projection + attention, interleaved emission ---
    # ch loop: KV + Q pack0; attention chunk (0,0) consumes each ch's
    # K/V/Q as soon as they are projected (kb block 4*ch needs ch's keys).
    po_cur = psO.tile([128, 2, 512], F32, tag="o", name="po")
    for ch in range(NCH):
        proj_kv(ch)
        vt_blocks(ch)
        proj_q(0, ch)
        attn_kbs(0, 0, po_cur, range(4 * ch, 4 * ch + 4))
    # pack-1 Q rope slots between the evac copy and the slow reciprocal on
    # the DVE queue so qt[1] is ready well before chunks (1,*).
    os_cur = finalize_evac(po_cur)
    proj_q(1, 0)
    finalize_norm(0, 0, os_cur)
    proj_q(1, 1)

    po_cur = psO.tile([128, 2, 512], F32, tag="o", name="po")
    attn_kbs(0, 1, po_cur, range(NKB))
    os_cur = finalize_evac(po_cur)
    proj_q(1, 2)
    finalize_norm(0, 1, os_cur)
    proj_q(1, 3)

    # remaining chunks; out-proj for a query block interleaves once its two
    # source chunks' normalizes are old (the Newton-chain costs are modeled
    # correctly, so the scheduler places these right).
    interleave = {
        (1, 2): [(qb, nch) for qb in range(0, 4) for nch in range(2)],
        (0, 3): [(qb, nch) for qb in range(4, 8) for nch in range(2)],
        (1, 3): [(qb, nch) for qb in range(8, 12) for nch in range(2)],
    }
    for (pair, cc) in [(1, 0), (1, 1), (0, 2), (1, 2), (0, 3), (1, 3)]:
        po_cur = psO.tile([128, 2, 512], F32, tag="o", name="po")
        units = interleave.get((pair, cc), [])
        p_last = None
        for g in range(4):
            p_last = attn_kbs(pair, cc, po_cur, range(4 * g, 4 * g + 4))
            for u in units[2 * g:2 * g + 2]:
                outproj_unit(*u)
        if (pair, cc) == (1, 3):
            # keep the PE clock warm through the final normalize chain; gate
            # the first filler on the last exp's P tile so the scheduler
            # places the burst inside the tail gap.
            wps = psT.tile([128, 512], F32, tag="pt", name="wps2")
            nc.tensor.matmul(
                wps, scratch[:, 0:128], p_last[:, 0, :], start=True, stop=True
            )
            for i in range(7):
                wps = psT.tile([128, 512], F32, tag="pt", name="wps2")
                nc.tensor.matmul(
                    wps, scratch[:, 0:128], scratch, start=True, stop=True
                )
            os_cur = finalize_evac(po_cur)
            # split the last normalize so qb12/13 gate on half 0 only
            finalize_norm(pair, cc, os_cur, half=0)
            for nch in range(2):
                outproj_unit(12, nch)
                outproj_unit(13, nch)
            finalize_norm(pair, cc, os_cur, half=1)
            for nch in range(2):
                outproj_unit(14, nch)
                outproj_unit(15, nch)
        else:
            os_cur = finalize_evac(po_cur)
            finalize_norm(pair, cc, os_cur)

    es.close()


def _rope_tables():
    inv_freq = 1.0 / (10000.0 ** (np.arange(0, D, 2, dtype=np.float64) / D))
    freqs = np.outer(np.arange(N, dtype=np.float64), inv_freq)  # [N, 32]
    cos_h = np.cos(freqs).astype(np.float32).T                  # [32, N]
    sin_h = np.sin(freqs).astype(np.float32).T                  # [32, N]
    cos128 = np.concatenate([cos_h] * 4, 0)                     # [128, N]
    # sin rows live at the SAME partitions as the rot-half source they are
    # multiplied with (walrus: SBUF-SBUF tensor_tensor inputs must share
    # base partition); the shifted write carries the rotation.
    sin128 = np.concatenate([sin_h, -sin_h, sin_h, -sin_h], 0)  # [128, N]
    return np.ascontiguousarray(cos128), np.ascontiguousarray(sin128)


@functools.lru_cache(maxsize=1)
def _program():
    nc = bacc.Bacc(
        "TRN2", target_bir_lowering=False, debug=False, enable_asserts=False
    )
    io = {
        "xt": nc.dram_tensor(
            "xt", [NCH, NDB, 128, 512], BF16, kind="ExternalInput"
        ).ap(),
        "wq": nc.dram_tensor(
            "wq", [2, NDB, 128, 128], BF16, kind="ExternalInput"
        ).ap(),
        "wkv": nc.dram_tensor(
            "wkv", [NDB, 128, 128], BF16, kind="ExternalInput"
        ).ap(),
        "wo": nc.dram_tensor("wo", [2, 128, DIM], BF16, kind="ExternalInput").ap(),
        "cosq": nc.dram_tensor("cosq", [128, N], BF16, kind="ExternalInput").ap(),
        "sinq": nc.dram_tensor("sinq", [128, N], BF16, kind="ExternalInput").ap(),
        "out": nc.dram_tensor("out", [N, DIM], F32, kind="ExternalOutput").ap(),
    }
    with tile.TileContext(nc) as tc:
        build_kernel(nc, tc, io)
    nc.compile()
    return nc


def make_in_maps(x, Wq, Wkv, Wo):
    import ml_dtypes

    bf16 = ml_dtypes.bfloat16
    cos128, sin128 = _rope_tables()
    in_maps = []
    for c in range(8):
        b, j = c // 4, c % 4
        # x[b].T [1024, 2048] -> [4ch, 8kb, 128, 512]
        xt = np.ascontiguousarray(
            x[b].T.reshape(NDB, 128, NCH, 512).transpose(2, 0, 1, 3)
        )
        # Wq cols for this core, pack-major [2, 8, 128, 128]
        wq_c = np.ascontiguousarray(
            Wq[:, 256 * j:256 * (j + 1)]
            .reshape(NDB, 128, 2, 128)
            .transpose(2, 0, 1, 3)
        )
        wkv_c = np.ascontiguousarray(
            np.concatenate(
                [Wkv[:, 64 * j:64 * (j + 1)],
                 Wkv[:, 256 + 64 * j:256 + 64 * (j + 1)]],
                axis=1,
            )
        ).reshape(NDB, 128, 128)
        wo_c = np.ascontiguousarray(Wo[256 * j:256 * (j + 1), :]).reshape(
            2, 128, DIM
        )
        in_maps.append(
            {
                "xt": xt.astype(bf16),
                "wq": wq_c.astype(bf16),
                "wkv": wkv_c.astype(bf16),
                "wo": wo_c.astype(bf16),
                "cosq": cos128.astype(bf16),
                "sinq": sin128.astype(bf16),
            }
        )
    return in_maps


def _install_ntff_hook():
    """Register the axon NTFF profiling hook that this image's antenv lacks."""
    import types

    if "antenv.axon_hooks" in sys.modules:
        return
    try:
        sys.path.append("/root/.axon_site")
        from trn_agent_boot.trn_boot import _ntff_profile_via_ctypes

        hook = _ntff_profile_via_ctypes("/opt/axon/libaxon_pjrt.so")
    except Exception:
        hook = None
    finally:
        try:
            sys.path.remove("/root/.axon_site")
        except ValueError:
            pass
    mod = types.ModuleType("antenv.axon_hooks")
    mod.get_axon_ntff_profile_hook = lambda: hook
    mod.set_axon_ntff_profile_hook = lambda h: None
    sys.modules["antenv.axon_hooks"] = mod
    # artifact upload needs bucket credentials this container lacks
    import concourse.bass_utils as bu

    bu.upload_artifacts = lambda tmpdir: "local://" + str(tmpdir)


def kernel(x, Wq, Wkv, Wo, bo):
    from concourse.bass_utils import run_bass_kernel_spmd

    _install_ntff_hook()
    nc = _program()
    in_maps = make_in_maps(x, Wq, Wkv, Wo)
    trace = bool(os.environ.get("KERNEL_TRACE"))
    res = run_bass_kernel_spmd(
        nc, in_maps, list(range(8)), trace=trace
    )
    LAST_RESULTS["res"] = res
    full = np.zeros((B, N, DIM), np.float32)
    for c in range(8):
        full[c // 4] += res.results[c]["out"]
    full += bo.astype(np.float32)
    return full
